# revision 12
# baseline (speedup 1.0000x reference)
"""Trainium2 Bass kernel for nn_ComplexMixture.

Per batch element b (R = input_real[b] [S,D], I = input_imag[b] [S,D], w [S]):
    out_r = (w*R)^T R + (w*I)^T I        (symmetric)
    out_i = (w*I)^T R - (w*R)^T I        (antisymmetric)

Host folds sqrt(w) into both operands (w >= 0):
    A = fp16(sqrt(w) * R),  B = fp16(sqrt(w) * I)
so   out_r = A^T A + B^T B,  out_i = B^T A - A^T B.

3-multiplication complex trick (25% fewer PE cycles than the 4-term form):
    P1 = A^T A,  P2 = B^T B,  P3 = (A-B)^T (A+B)
    out_r = P1 + P2
    out_i = P3 + P2 - P1        (P3 = P1 - P2 + A^T B - B^T A ... sign check:
                                 (A-B)^T(A+B) = A^TA + A^TB - B^TA - B^TB,
                                 so P3 + P2 - P1 = A^TB - B^TA = -out_i;
                                 we therefore compute oi = -(P3 + P2 - P1) by
                                 swapping: oi = P1 - P2 - P3 ... see code: we
                                 use E = B - A, F = B + A so that
                                 E^TF = B^TB + B^TA - A^TB - A^TA and
                                 E^TF + P1 - P2 = B^TA - A^TB = out_i.)

out_r is symmetric and out_i antisymmetric, so only the upper block-trapezoid
is computed: row tile m (128 rows) covers columns [128m, 768) (width 768-128m),
chopped into <=512-column chunks (PSUM bank limit). 96 matmuls/core instead of
the 4-term full-square 192.

Evacuation per row tile m:
    ScalarE:  u1 = copy(P1)              (PSUM -> SBUF fp32)
    VectorE:  or = P2 + u1               (-> SBUF fp16, stored)
              t1 = P2 - u1               (-> SBUF fp32)   [= P2 - P1]
              oi = P3 - t1?  no: oi = E^TF + (P1 - P2) = P3' - t1
              implemented as tensor_sub(oi, P3', t1)      (-> SBUF fp16)
with E = B - A, F = B + A computed on VectorE from the fp16 inputs.

Sharding: data-parallel over batch, one batch element per core (B == 8).
Outputs are stored fp16 (halves store traffic); the host upcasts to fp32 and
mirrors the skipped strictly-lower blocks (transpose / negated transpose --
exact float ops).

A short prewarm burst of dummy matmuls bridges the PE p-state ramp while the
first input DMAs land.
"""

import sys
import types

import numpy as np

# If the environment requests tracing (BASS_TRACE=1) but the image lacks
# antenv.axon_hooks, bass_utils would crash importing it; provide a no-op
# hook registry so tracing degrades gracefully instead.
try:
    import antenv.axon_hooks  # noqa: F401
except ImportError:
    _hooks = types.ModuleType("antenv.axon_hooks")
    _hooks._hook = None
    _hooks.set_axon_ntff_profile_hook = lambda h: setattr(_hooks, "_hook", h)
    _hooks.get_axon_ntff_profile_hook = lambda: _hooks._hook
    sys.modules["antenv.axon_hooks"] = _hooks

import concourse.bacc as bacc
import concourse.bass_utils as bass_utils
import concourse.mybir as mybir
import concourse.tile as tile

B, S, D = 8, 512, 768
P = 128          # SBUF/PSUM partitions; matmul contraction tile
KC = S // P      # 4 contraction chunks
MT = D // P      # 6 output row tiles
N_CORES = 8
N_PREWARM = 9    # dummy N=512 matmuls bridging the PE p-state ramp while the
                 # input tensors are in flight (~4us of DMA)

# Row tile m covers columns [128m, 768), split into <=512 chunks.
CHUNKS = {}
for _m in range(MT):
    _c0 = _m * P
    if D - _c0 > 512:
        CHUNKS[_m] = [(_c0, _c0 + 512), (_c0 + 512, D)]
    else:
        CHUNKS[_m] = [(_c0, D)]

# Packed-trapezoid output layout: row tile m's [128, 768-128m] strip lives at
# column offset TRAP_OFF[m] of a [P, TRAP_W] tensor. Large per-partition DMA
# segments (1.5-5KB) get far better per-queue DMA throughput than per-m
# strips would.
TRAP_OFF = [0]
for _m in range(MT):
    TRAP_OFF.append(TRAP_OFF[-1] + D - _m * P)
TRAP_W = TRAP_OFF[-1]  # 2688

# Manual PSUM bank rotation (8 banks, tags bk0..bk7). Chosen so each bank's
# next writer starts well after its previous reader finished (see transcript
# analysis): P1 is scalar-copied out early; P2 freed by the or/t1 passes; P3
# freed last by the oi pass and gets the longest reuse distance.
BANKS = {
    0: {"P1": (0, 1), "P2": (2, 3), "P3": (4, 5)},   # m0: (chunk_a, chunk_b)
    1: {"P1": (6, 7), "P2": (0, 1), "P3": (2, 3)},   # m1
    2: {"P1": (4,), "P2": (5,), "P3": (6,)},          # m2
    3: {"P1": (7,), "P2": (0,), "P3": (1,)},          # m3
    4: {"P1": (2,), "P2": (3,), "P3": (4,)},          # m4
    5: {"P1": (5,), "P2": (6,), "P3": (7,)},          # m5
}

_CACHE: dict = {}


def _build():
    f32, f16 = mybir.dt.float32, mybir.dt.float16
    nc = bacc.Bacc(
        "TRN2", target_bir_lowering=False, debug=False, num_devices=N_CORES
    )
    # Host-packed partition-major: a_in[p, k*D:(k+1)*D] = A[k*P+p, :].
    a_d = nc.dram_tensor("a_in", [P, KC * D], f16, kind="ExternalInput").ap()
    b_d = nc.dram_tensor("b_in", [P, KC * D], f16, kind="ExternalInput").ap()
    # packed upper trapezoid (see TRAP_OFF); host unpacks + mirrors
    or_d = nc.dram_tensor("or_out", [P, TRAP_W], f16, kind="ExternalOutput").ap()
    oi_d = nc.dram_tensor("oi_out", [P, TRAP_W], f16, kind="ExternalOutput").ap()

    with tile.TileContext(nc) as tc:
        with (
            tc.tile_pool(name="const", bufs=1) as cpool,
            tc.tile_pool(name="stage", bufs=1) as spool,
            tc.tile_pool(name="ef", bufs=1) as epool,
            tc.tile_pool(name="osb", bufs=2) as opool,
            tc.tile_pool(name="ps", bufs=1, space="PSUM") as pspool,
        ):
            # PE prewarm on zeros: starts the p-state ramp while input DMAs
            # are in flight. Lands in bank 6 (first real use: m1's P1a, ~5us
            # later).
            zw = cpool.tile([P, 5 * P], f16, name="zw")
            nc.vector.memset(zw[:], 0.0)
            pw_ps = pspool.tile([P, 512], f32, name="pw_ps", tag="bk6")
            for _ in range(N_PREWARM):
                nc.tensor.matmul(
                    pw_ps[:], zw[:, 0:P], zw[:, P : 5 * P], start=True, stop=True
                )

            # Inputs as one whole-tensor DMA per ring: 6KB per-partition
            # segments get ~190GB/s per queue (vs ~65GB/s for 1.5KB rows).
            # a rides the sync ring (starts ~1.5us earlier than scalar), so
            # the a-only P1 streams below can begin before b lands.
            a_t = spool.tile([P, KC * D], f16, name="a_t", tag="a_t")
            b_t = spool.tile([P, KC * D], f16, name="b_t", tag="b_t")
            nc.sync.dma_start(a_t[:], a_d[:])
            nc.scalar.dma_start(b_t[:], b_d[:])

            def asl(k, c0, c1):
                return a_t[:, k * D + c0 : k * D + c1]

            def bsl(k, c0, c1):
                return b_t[:, k * D + c0 : k * D + c1]

            # E = B - A, F = B + A (so E^TF + P1 - P2 = out_i directly).
            # Split across VectorE and GpSimd so the last pair is ready
            # before the P3 streams reach it.
            et, ft = [], []
            for k in range(KC):
                e = epool.tile([P, D], f16, name=f"e{k}", tag=f"e{k}")
                f = epool.tile([P, D], f16, name=f"f{k}", tag=f"f{k}")
                eng = nc.gpsimd if k >= 2 else nc.vector
                eng.tensor_sub(e[:], bsl(k, 0, D), asl(k, 0, D))
                feng = nc.gpsimd if k == 2 else nc.vector
                feng.tensor_add(f[:], bsl(k, 0, D), asl(k, 0, D))
                et.append(e)
                ft.append(f)

            # Packed-trapezoid staging: all m strips side by side, so the
            # store DMAs have large contiguous per-partition segments.
            or_pk = opool.tile([P, TRAP_W], f16, name="or_pk", tag="or_pk")
            oi_pk = opool.tile([P, TRAP_W], f16, name="oi_pk", tag="oi_pk")

            for m in range(MT):
                ms0, ms1 = m * P, (m + 1) * P
                chunks = CHUNKS[m]
                bk = BANKS[m]
                ps1 = [
                    pspool.tile([P, 512], f32, name=f"p1_{m}_{ci}", tag=f"bk{bk['P1'][ci]}")
                    for ci in range(len(chunks))
                ]
                ps2 = [
                    pspool.tile([P, 512], f32, name=f"p2_{m}_{ci}", tag=f"bk{bk['P2'][ci]}")
                    for ci in range(len(chunks))
                ]
                ps3 = [
                    pspool.tile([P, 512], f32, name=f"p3_{m}_{ci}", tag=f"bk{bk['P3'][ci]}")
                    for ci in range(len(chunks))
                ]

                def mm1(ci, k):
                    ca, cb = chunks[ci]
                    nc.tensor.matmul(
                        ps1[ci][:, 0 : cb - ca], asl(k, ms0, ms1), asl(k, ca, cb),
                        start=(k == 0), stop=(k == KC - 1),
                    )

                def mm2(ci, k):
                    ca, cb = chunks[ci]
                    nc.tensor.matmul(
                        ps2[ci][:, 0 : cb - ca], bsl(k, ms0, ms1), bsl(k, ca, cb),
                        start=(k == 0), stop=(k == KC - 1),
                    )

                def mm3(ci, k):
                    ca, cb = chunks[ci]
                    nc.tensor.matmul(
                        ps3[ci][:, 0 : cb - ca], et[k][:, ms0:ms1], ft[k][:, ca:cb],
                        start=(k == 0), stop=(k == KC - 1),
                    )

                # stream-outer: P1 (a-only) runs while b is still landing;
                # P1 also finishes early so its bank turns over fast
                # (scalar copy), P2 next, P3 last.
                for k in range(KC):
                    for ci in range(len(chunks)):
                        mm1(ci, k)
                for k in range(KC):
                    for ci in range(len(chunks)):
                        mm2(ci, k)
                for k in range(KC):
                    for ci in range(len(chunks)):
                        mm3(ci, k)

                # Evacuate: u1 = P1 (ScalarE, frees P1's banks early),
                # or = P2 + u1, t1 = P2 - u1, oi = P3 - t1 (VectorE).
                u1 = opool.tile([P, D], f32, name=f"u1_{m}", tag="u1")
                t1 = opool.tile([P, D], f32, name=f"t1_{m}", tag="t1")
                toff = TRAP_OFF[m] - ms0
                for ci, (ca, cb) in enumerate(chunks):
                    o0, o1 = ca - ms0, cb - ms0
                    nc.scalar.copy(u1[:, o0:o1], ps1[ci][:, 0 : cb - ca])
                for ci, (ca, cb) in enumerate(chunks):
                    nc.vector.tensor_add(
                        or_pk[:, toff + ca : toff + cb], ps2[ci][:, 0 : cb - ca],
                        u1[:, ca - ms0 : cb - ms0],
                    )
                    nc.vector.tensor_sub(
                        t1[:, ca - ms0 : cb - ms0], ps2[ci][:, 0 : cb - ca],
                        u1[:, ca - ms0 : cb - ms0],
                    )
                for ci, (ca, cb) in enumerate(chunks):
                    nc.vector.tensor_sub(
                        oi_pk[:, toff + ca : toff + cb], ps3[ci][:, 0 : cb - ca],
                        t1[:, ca - ms0 : cb - ms0],
                    )
                # Store in three large slabs per output: after m0 (cols
                # 0:768), after m2 (768:1920), after m5 (1920:2688).
                if m in (0, 2, 5):
                    s0, s1 = TRAP_OFF[0 if m == 0 else (1 if m == 2 else 3)], TRAP_OFF[m + 1]
                    nc.sync.dma_start(or_d[:, s0:s1], or_pk[:, s0:s1])
                    nc.scalar.dma_start(oi_d[:, s0:s1], oi_pk[:, s0:s1])

    nc.compile()
    return nc


def get_nc():
    if "nc" not in _CACHE:
        _CACHE["nc"] = _build()
    return _CACHE["nc"]


def make_in_maps(input_real, input_imag, weight):
    input_real = np.asarray(input_real, dtype=np.float32)
    input_imag = np.asarray(input_imag, dtype=np.float32)
    weight = np.asarray(weight, dtype=np.float32)
    sq = np.sqrt(weight)[:, :, None]  # [B, S, 1]
    a = (sq * input_real).astype(np.float16)
    b = (sq * input_imag).astype(np.float16)
    # pack [S, D] -> [P, KC*D]: row p holds chunks k=0..KC-1 concatenated
    a = a.reshape(B, KC, P, D).transpose(0, 2, 1, 3).reshape(B, P, KC * D)
    b = b.reshape(B, KC, P, D).transpose(0, 2, 1, 3).reshape(B, P, KC * D)
    return [
        {
            "a_in": np.ascontiguousarray(a[i]),
            "b_in": np.ascontiguousarray(b[i]),
        }
        for i in range(B)
    ]


def run(input_real, input_imag, weight, **spmd_kwargs):
    nc = get_nc()
    res = bass_utils.run_bass_kernel_spmd(
        nc,
        make_in_maps(input_real, input_imag, weight),
        core_ids=list(range(N_CORES)),
        **spmd_kwargs,
    )
    or_pk = np.stack([res.results[i]["or_out"] for i in range(B)]).astype(np.float32)
    oi_pk = np.stack([res.results[i]["oi_out"] for i in range(B)]).astype(np.float32)
    # Unpack the trapezoid strips, then mirror the skipped strictly-lower
    # blocks: out_r symmetric, out_i antisymmetric (exact float ops).
    out_r = np.empty((B, D, D), np.float32)
    out_i = np.empty((B, D, D), np.float32)
    for m in range(MT):
        c0, o0 = m * P, TRAP_OFF[m]
        out_r[:, c0 : c0 + P, c0:D] = or_pk[:, :, o0 : o0 + D - c0]
        out_i[:, c0 : c0 + P, c0:D] = oi_pk[:, :, o0 : o0 + D - c0]
    vr = out_r.reshape(B, MT, P, MT, P)
    vi = out_i.reshape(B, MT, P, MT, P)
    for bi in range(1, MT):
        for bj in range(bi):
            vr[:, bi, :, bj, :] = vr[:, bj, :, bi, :].transpose(0, 2, 1)
            vi[:, bi, :, bj, :] = -vi[:, bj, :, bi, :].transpose(0, 2, 1)
    return (out_r, out_i), res


def kernel(input_real, input_imag, weight):
    (out_r, out_i), _ = run(input_real, input_imag, weight)
    return (out_r, out_i)
